# revision 9
# baseline (speedup 1.0000x reference)
"""Causal multi-head attention with RoPE on 8 Trainium2 NeuronCores.

Reference computation (fp32):
    qkv = x @ Wqkv.T ; split q,k,v ; heads 16 x 64 ; interleaved-pair RoPE on
    q,k ; causal softmax(q k^T / 8) @ v ; concat heads ; out @ Wout.T

Sharding: core c -> batch b=c//2, head-group g=c%2 (heads 8g..8g+8).
Each core computes a [2048, 1024] partial of the output projection for its
batch (contraction over its 512 head-dims); host sums core pairs.

Kernel-internal layout tricks:
  - Wqkv rows per head are permuted evens-then-odds so RoPE becomes
    block-wise (no interleaving on device). The same permutation applied to
    q and k leaves q.k^T invariant.
  - Scores are computed transposed (S^T[k, q]) so the PV matmul needs no
    transposes; PV uses a ones-augmented V (M=65) so row 64 of the PV psum
    accumulates the softmax denominator for free.
  - Denominators are transposed via the PE, reciprocated, and broadcast
    across partitions with a ones @ diag(recip) matmul; the division is a
    single fused elementwise multiply on the attention output.
"""

import math
import sys

import numpy as np

sys.path.insert(0, "/opt/trn_rl_repo")

import concourse.bass as bass  # noqa: E402
import concourse.mybir as mybir  # noqa: E402
from concourse import bacc, tile  # noqa: E402
from concourse.masks import make_identity  # noqa: E402

D_MODEL = 1024
NUM_HEADS = 16
DH = 64
S = 2048
B = 4
THETA = 10000.0
P = 128
N_CORES = 8
F = 512  # free-dim chunk
N_SC = S // F  # 4 s-chunks
N_QT = S // P  # 16 q-tiles of 128
HPAIRS = 4  # head pairs per core
NEG = -1.0e30

# matmul input dtype for the big matmuls: float32 (4 cyc/row, exact) or
# float32r (1 cyc/row at N>=256, reduced precision).
import os  # noqa: E402

MM_DT = getattr(mybir.dt, os.environ.get("MM_DT", "float32r"))




def build_program(debug: bool = False):
    """Build the single-core SPMD program. Returns (nc, io_names)."""
    nc = bacc.Bacc("TRN2", target_bir_lowering=False, debug=debug,
                   enable_asserts=debug)
    f32 = mybir.dt.float32

    xt_d = nc.dram_tensor("xt", [D_MODEL, S], f32, kind="ExternalInput")
    wq_d = nc.dram_tensor("wqkv", [D_MODEL, 12 * P], f32, kind="ExternalInput")
    wo_d = nc.dram_tensor("wout", [4 * P, D_MODEL], f32, kind="ExternalInput")
    cos_d = nc.dram_tensor("costab", [P, S], f32, kind="ExternalInput")
    sin_d = nc.dram_tensor("sintab", [P, S], f32, kind="ExternalInput")
    mask_d = nc.dram_tensor("masks", [4, P, F], f32, kind="ExternalInput")
    out_d = nc.dram_tensor("out", [S, D_MODEL], f32, kind="ExternalOutput")

    xt_r = xt_d.ap().rearrange("(dc p) s -> p dc s", p=P)  # [128, 8, 2048]
    wq_r = wq_d.ap().rearrange("(dc p) n -> p dc n", p=P)  # [128, 8, 1536]
    wo_r = wo_d.ap().rearrange("(hp p) e -> p hp e", p=P)  # [128, 4, 1024]
    mask_r = mask_d.ap().rearrange("m p n -> p m n")       # [128, 4, 512]

    cdt = MM_DT
    with tile.TileContext(nc) as tc:
        with (
            tc.tile_pool(name="const", bufs=1) as const,
            tc.tile_pool(name="wq", bufs=2) as wqp,
            tc.tile_pool(name="xt", bufs=2) as xtp,
            tc.tile_pool(name="qkv", bufs=1) as qkvp,
            tc.tile_pool(name="tmp", bufs=2) as tmpp,
            tc.tile_pool(name="outt", bufs=1) as outtp,
            tc.tile_pool(name="exp", bufs=4) as expp,
            tc.tile_pool(name="fin", bufs=3) as finp,
            tc.tile_pool(name="small", bufs=2) as smallp,
            tc.tile_pool(name="psb", bufs=6, space="PSUM") as psb,
            tc.tile_pool(name="pss", bufs=2, space="PSUM") as pss,
        ):
            # ---- constants ----
            ident = const.tile([P, P], f32)
            make_identity(nc, ident)
            ones64 = const.tile([P, 64], f32)
            nc.vector.memset(ones64[:], 1.0)
            cost = const.tile([P, S], f32)
            nc.sync.dma_start(cost[:], cos_d.ap())
            sint = const.tile([P, S], f32)
            nc.sync.dma_start(sint[:], sin_d.ap())
            maskt = const.tile([P, 4, F], f32)
            nc.sync.dma_start(maskt[:], mask_r)
            woutt = const.tile([P, 4, D_MODEL], cdt)
            nc.gpsimd.dma_start(woutt[:], wo_r)
            # attention output (d-major), all 4 head pairs: rows=[hA|hB] dims
            outt = outtp.tile([P, HPAIRS, S], cdt)
            # softmax denominators: heads 0..3 on rows {0,32,64,96} of rsA,
            # heads 4..7 on rsB (engine APs need 32-aligned start partitions)
            rsA = const.tile([P, S], f32, name="rsA")
            rsB = const.tile([P, S], f32, name="rsB")
            nc.vector.memset(rsA[:], 1.0)
            nc.vector.memset(rsB[:], 1.0)
            # reciprocals, transposed: [q-within-tile, qtile, head]
            rcp = const.tile([P, N_QT, 8], f32)

            for hp in range(HPAIRS):
                whp = wqp.tile([P, 8, 3 * P], cdt)
                nc.gpsimd.dma_start(whp[:], wq_r[:, :, hp * 3 * P:(hp + 1) * 3 * P])
                q_rot = qkvp.tile([P, S], cdt, tag="q_rot")
                k_rot = qkvp.tile([P, S], cdt, tag="k_rot")
                # V s-major + ones cols: [s-part, ktile, (vA|1|vB|1)]
                v_sb = qkvp.tile([P, N_QT, 130], cdt, tag="v_sb")
                nc.vector.tensor_copy(v_sb[:, :, 64:65], ones64[:, 0:N_QT, None])
                nc.vector.tensor_copy(v_sb[:, :, 129:130], ones64[:, 0:N_QT, None])

                for sc in range(N_SC):
                    xts = xtp.tile([P, 8, F], cdt)
                    nc.gpsimd.dma_start(xts[:], xt_r[:, :, sc * F:(sc + 1) * F])
                    sl = slice(sc * F, (sc + 1) * F)
                    # q and k groups (d-major) with fused RoPE on psum-evac
                    for gi, dst in ((0, q_rot), (1, k_rot)):
                        ps = psb.tile([P, F], f32, tag="big")
                        for dc in range(8):
                            nc.tensor.matmul(
                                ps[:],
                                whp[:, dc, gi * P:(gi + 1) * P],
                                xts[:, dc, :],
                                start=(dc == 0), stop=(dc == 7),
                            )
                        # rot = ps*cos + swap_within_head(ps)*sins
                        tcs = tmpp.tile([P, F], f32, tag="ropetmp")
                        nc.vector.tensor_tensor(tcs[:], ps[:], cost[:, sl],
                                                mybir.AluOpType.mult)
                        for h2 in (0, 64):
                            nc.vector.tensor_tensor(
                                dst[h2:h2 + 32, sl], ps[h2 + 32:h2 + 64, :],
                                sint[h2:h2 + 32, sl], mybir.AluOpType.mult)
                            nc.vector.tensor_tensor(
                                dst[h2 + 32:h2 + 64, sl], ps[h2:h2 + 32, :],
                                sint[h2 + 32:h2 + 64, sl], mybir.AluOpType.mult)
                        nc.vector.tensor_tensor(dst[:, sl], dst[:, sl], tcs[:],
                                                mybir.AluOpType.add)
                    # v group: d-major matmul, then PE-transpose to s-major
                    ps = psb.tile([P, F], f32, tag="big")
                    for dc in range(8):
                        nc.tensor.matmul(
                            ps[:], whp[:, dc, 2 * P:3 * P],
                            xts[:, dc, :], start=(dc == 0), stop=(dc == 7),
                        )
                    vdm = tmpp.tile([P, F], f32, tag="vdm")
                    nc.vector.tensor_copy(vdm[:], ps[:])
                    for j in range(4):
                        kt = sc * 4 + j
                        pt = pss.tile([P, P], f32, tag="small")
                        nc.tensor.transpose(pt[:], vdm[:, j * P:(j + 1) * P],
                                            ident[:])
                        nc.vector.tensor_copy(v_sb[:, kt, 0:64], pt[:, 0:64])
                        nc.vector.tensor_copy(v_sb[:, kt, 65:129], pt[:, 64:128])

                # ---- causal attention for this head pair ----
                for qc in range(N_SC):
                    qsl = slice(qc * F, (qc + 1) * F)
                    po = [psb.tile([P, F], f32, tag="big", name=f"po{h2}")
                          for h2 in range(2)]
                    nkt = 4 * qc + 4
                    for kt in range(nkt):
                        for h2 in (0, 1):
                            base = 64 * h2
                            ps = psb.tile([P, F], f32, tag="big")
                            nc.tensor.matmul(
                                ps[:],
                                k_rot[base:base + 64, kt * P:(kt + 1) * P],
                                q_rot[base:base + 64, qsl],
                                start=True, stop=True,
                            )
                            if kt >= 4 * qc:
                                nc.vector.tensor_tensor(
                                    ps[:], ps[:], maskt[:, kt - 4 * qc, :],
                                    mybir.AluOpType.add)
                            ex = expp.tile([P, F], cdt)
                            nc.scalar.activation(
                                ex[:], ps[:], mybir.ActivationFunctionType.Exp,
                                scale=1.0 / math.sqrt(DH))
                            nc.tensor.matmul(
                                po[h2][0:65, :],
                                v_sb[:, kt, 65 * h2:65 * h2 + 65],
                                ex[:],
                                start=(kt == 0), stop=(kt == nkt - 1),
                            )
                    for h2 in (0, 1):
                        nc.vector.tensor_copy(
                            outt[64 * h2:64 * h2 + 64, hp, qsl],
                            po[h2][0:64, :])
                        h = 2 * hp + h2
                        rsX = rsA if h < 4 else rsB
                        row = 32 * (h % 4)
                        nc.vector.tensor_copy(rsX[row:row + 1, qsl],
                                              po[h2][64:65, :])

            # ---- denominators: transpose, reciprocal, broadcast, divide ----
            for j in range(N_QT):
                for half, rsX in ((0, rsA), (1, rsB)):
                    pt = pss.tile([P, P], f32, tag="small")
                    nc.tensor.transpose(pt[:], rsX[:, j * P:(j + 1) * P],
                                        ident[:])
                    for hh in range(4):
                        h = 4 * half + hh
                        nc.vector.reciprocal(rcp[:, j, h:h + 1],
                                             pt[:, 32 * hh:32 * hh + 1])
            for hp in range(HPAIRS):
                for j in range(N_QT):
                    jsl = slice(j * P, (j + 1) * P)
                    for h2 in (0, 1):
                        h = 2 * hp + h2
                        diag = smallp.tile([P, P], f32, tag="diag")
                        nc.vector.tensor_scalar(
                            diag[:], ident[:], rcp[:, j, h:h + 1], None,
                            mybir.AluOpType.mult)
                        pbc = pss.tile([P, P], f32, tag="small")
                        nc.tensor.matmul(pbc[0:64, :], ones64[:, 0:64],
                                         diag[:], start=True, stop=True)
                        nc.vector.tensor_tensor(
                            outt[64 * h2:64 * h2 + 64, hp, jsl],
                            outt[64 * h2:64 * h2 + 64, hp, jsl],
                            pbc[0:64, :], mybir.AluOpType.mult)

            # ---- output projection: natural [s, e] partial ----
            for ec in range(2):
                esl = slice(ec * F, (ec + 1) * F)
                for st in range(N_QT):
                    pf = psb.tile([P, F], f32, tag="big")
                    for hp in range(HPAIRS):
                        nc.tensor.matmul(
                            pf[:], outt[:, hp, st * P:(st + 1) * P],
                            woutt[:, hp, esl],
                            start=(hp == 0), stop=(hp == 3),
                        )
                    fo = finp.tile([P, F], f32)
                    nc.vector.tensor_copy(fo[:], pf[:])
                    nc.sync.dma_start(
                        out_d.ap()[st * P:(st + 1) * P, esl], fo[:])

    nc.compile()
    return nc


def _rope_tables():
    k = np.arange(DH // 2, dtype=np.float64)
    invf = THETA ** (-2.0 * k / DH)
    pos = np.arange(S, dtype=np.float64)
    ang = invf[:, None] * pos[None, :]  # [32, S]
    cos32 = np.cos(ang)
    sin32 = np.sin(ang)
    cos = np.tile(cos32, (4, 1)).astype(np.float32)          # [128, S]
    sins = np.concatenate([-sin32, sin32, -sin32, sin32], 0).astype(np.float32)
    return cos, sins


def _masks():
    m = np.arange(4)[:, None, None]
    i = np.arange(P)[None, :, None]
    j = np.arange(F)[None, None, :]
    return np.where(P * m + i > j, np.float32(NEG), np.float32(0.0))


def host_inputs(x, Wqkv, Wout, core):
    """Per-core input dict (all fp32 contiguous)."""
    b, g = core // 2, core % 2
    xt = np.ascontiguousarray(x[b].T, dtype=np.float32)  # [1024, 2048]
    perm = np.concatenate([np.arange(0, DH, 2), np.arange(1, DH, 2)])
    blocks = []
    for hp in range(HPAIRS):
        hA = 8 * g + 2 * hp
        for off, do_perm in ((0, True), (D_MODEL, True), (2 * D_MODEL, False)):
            for h in (hA, hA + 1):
                rows = Wqkv[off + h * DH: off + (h + 1) * DH]
                if do_perm:
                    rows = rows[perm]
                blocks.append(rows)
    wq = np.ascontiguousarray(np.concatenate(blocks, 0).T, dtype=np.float32)
    wo = np.ascontiguousarray(Wout[:, 512 * g:512 * (g + 1)].T,
                              dtype=np.float32)
    cos, sins = _rope_tables()
    return {"xt": xt, "wqkv": wq, "wout": wo, "costab": cos, "sintab": sins,
            "masks": _masks()}


_CACHE = {}


def kernel(x, Wqkv, Wout):
    from concourse.bass_utils import run_bass_kernel_spmd

    x = np.asarray(x, dtype=np.float32)
    Wqkv = np.asarray(Wqkv, dtype=np.float32)
    Wout = np.asarray(Wout, dtype=np.float32)

    if "nc" not in _CACHE:
        _CACHE["nc"] = build_program(debug=False)
    nc = _CACHE["nc"]

    in_maps = [host_inputs(x, Wqkv, Wout, c) for c in range(N_CORES)]
    res = run_bass_kernel_spmd(nc, in_maps, list(range(N_CORES))).results
    out = np.empty((B, S, D_MODEL), dtype=np.float32)
    for b in range(B):
        out[b] = res[2 * b]["out"] + res[2 * b + 1]["out"]
    return out


# revision 12
# speedup vs baseline: 1.1395x; 1.1395x over previous
"""Causal multi-head attention with RoPE on 8 Trainium2 NeuronCores.

Reference computation (fp32):
    qkv = x @ Wqkv.T ; split q,k,v ; heads 16 x 64 ; interleaved-pair RoPE on
    q,k ; causal softmax(q k^T / 8) @ v ; concat heads ; out @ Wout.T

Sharding: core c -> batch b=c//2, head-group g=c%2 (heads 8g..8g+8).
Each core computes a [2048, 1024] partial of the output projection for its
batch (contraction over its 512 head-dims); host sums core pairs.

Kernel-internal layout tricks:
  - Wqkv rows per head are permuted evens-then-odds so RoPE becomes
    block-wise (no interleaving on device). The same permutation applied to
    q and k leaves q.k^T invariant.
  - Scores are computed transposed (S^T[k, q]) so the PV matmul needs no
    transposes; PV uses a ones-augmented V (M=65) so row 64 of the PV psum
    accumulates the softmax denominator for free.
  - Denominators are transposed via the PE, reciprocated, and broadcast
    across partitions with a ones @ diag(recip) matmul; the division is a
    single fused elementwise multiply on the attention output.
"""

import math
import sys

import numpy as np

sys.path.insert(0, "/opt/trn_rl_repo")

import concourse.bass as bass  # noqa: E402
import concourse.mybir as mybir  # noqa: E402
from concourse import bacc, tile  # noqa: E402
from concourse.masks import make_identity  # noqa: E402

D_MODEL = 1024
NUM_HEADS = 16
DH = 64
S = 2048
B = 4
THETA = 10000.0
P = 128
N_CORES = 8
F = 512  # free-dim chunk
N_SC = S // F  # 4 s-chunks
N_QT = S // P  # 16 q-tiles of 128
HPAIRS = 4  # head pairs per core
NEG = -1.0e30

# matmul input dtype for the big matmuls: float32 (4 cyc/row, exact) or
# float32r (1 cyc/row at N>=256, reduced precision).
import os  # noqa: E402

MM_DT = getattr(mybir.dt, os.environ.get("MM_DT", "float32r"))




def build_program(debug: bool = False):
    """Build the single-core SPMD program. Returns (nc, io_names)."""
    nc = bacc.Bacc("TRN2", target_bir_lowering=False, debug=debug,
                   enable_asserts=debug)
    f32 = mybir.dt.float32

    cdt = MM_DT
    xt_d = nc.dram_tensor("xt", [D_MODEL, S], cdt, kind="ExternalInput")
    wq_d = nc.dram_tensor("wqkv", [D_MODEL, 12 * P], cdt, kind="ExternalInput")
    wo_d = nc.dram_tensor("wout", [4 * P, D_MODEL], cdt, kind="ExternalInput")
    cos_d = nc.dram_tensor("costab", [P, S], f32, kind="ExternalInput")
    sin_d = nc.dram_tensor("sintab", [P, S], f32, kind="ExternalInput")
    mask_d = nc.dram_tensor("masks", [4, P, F], cdt, kind="ExternalInput")
    out_d = nc.dram_tensor("out", [S, D_MODEL], f32, kind="ExternalOutput")

    xt_r = xt_d.ap().rearrange("(dc p) s -> p dc s", p=P)  # [128, 8, 2048]
    wq_r = wq_d.ap().rearrange("(dc p) n -> p dc n", p=P)  # [128, 8, 1536]
    wo_r = wo_d.ap().rearrange("(hp p) e -> p hp e", p=P)  # [128, 4, 1024]
    mask_r = mask_d.ap().rearrange("m p n -> p m n")       # [128, 4, 512]

    with tile.TileContext(nc) as tc:
        with (
            tc.tile_pool(name="const", bufs=1) as const,
            tc.tile_pool(name="wq", bufs=2) as wqp,
            tc.tile_pool(name="xt", bufs=2) as xtp,
            tc.tile_pool(name="qkv", bufs=1) as qkvp,
            tc.tile_pool(name="tmp", bufs=2) as tmpp,
            tc.tile_pool(name="outt", bufs=1) as outtp,
            tc.tile_pool(name="exp", bufs=4) as expp,
            tc.tile_pool(name="fin", bufs=3) as finp,
            tc.tile_pool(name="small", bufs=2) as smallp,
            tc.tile_pool(name="psb", bufs=3, space="PSUM") as psb,
            tc.tile_pool(name="pss", bufs=2, space="PSUM") as pss,
        ):
            # ---- constants ----
            ident = const.tile([P, P], f32)
            make_identity(nc, ident)
            ones64 = const.tile([P, 64], f32)
            nc.vector.memset(ones64[:], 1.0)
            identc = const.tile([P, P], cdt)
            nc.vector.tensor_copy(identc[:], ident[:])
            cost = const.tile([P, S], f32)
            nc.sync.dma_start(cost[:], cos_d.ap())
            sint = const.tile([P, S], f32)
            nc.sync.dma_start(sint[:], sin_d.ap())
            maskt = const.tile([P, 4, F], cdt)
            nc.sync.dma_start(maskt[:], mask_r)
            woutt = const.tile([P, 4, D_MODEL], cdt)
            nc.sync.dma_start(woutt[:], wo_r)
            # attention output (d-major), all 4 head pairs: rows=[hA|hB] dims
            outt = outtp.tile([P, HPAIRS, S], cdt)
            # softmax denominators: heads 0..3 on rows {0,32,64,96} of rsA,
            # heads 4..7 on rsB (engine APs need 32-aligned start partitions)
            rsA = const.tile([P, S], f32, name="rsA")
            rsB = const.tile([P, S], f32, name="rsB")
            nc.vector.memset(rsA[:], 1.0)
            nc.vector.memset(rsB[:], 1.0)
            # reciprocals, transposed: [q-within-tile, qtile, head]
            rcp = const.tile([P, N_QT, 8], f32)

            for hp in range(HPAIRS):
                whp = wqp.tile([P, 8, 3 * P], cdt)
                nc.sync.dma_start(whp[:], wq_r[:, :, hp * 3 * P:(hp + 1) * 3 * P])
                q_rot = qkvp.tile([P, S], cdt, tag="q_rot")
                k_rot = qkvp.tile([P, S], cdt, tag="k_rot")
                # V s-major + ones cols: [s-part, ktile, (vA|1|vB|1)]
                v_sb = qkvp.tile([P, N_QT, 130], cdt, tag="v_sb")
                nc.vector.tensor_copy(v_sb[:, :, 64:65], ones64[:, 0:N_QT, None])
                nc.vector.tensor_copy(v_sb[:, :, 129:130], ones64[:, 0:N_QT, None])

                for sc in range(N_SC):
                    xts = xtp.tile([P, 8, F], cdt)
                    nc.sync.dma_start(xts[:], xt_r[:, :, sc * F:(sc + 1) * F])
                    sl = slice(sc * F, (sc + 1) * F)
                    # q and k groups (d-major) with fused RoPE on psum-evac
                    for gi, dst in ((0, q_rot), (1, k_rot)):
                        psw = psb.tile([P, 2 * F], f32, tag="spair", bufs=2,
                                       name="psw")
                        ps = psw[:, 0:F]
                        for dc in range(8):
                            nc.tensor.matmul(
                                ps[:],
                                whp[:, dc, gi * P:(gi + 1) * P],
                                xts[:, dc, :],
                                start=(dc == 0), stop=(dc == 7),
                            )
                        # rot = ps*cos + swap_within_head(ps)*sins
                        tcs = tmpp.tile([P, F], f32, tag="ropetmp")
                        nc.vector.tensor_tensor(tcs[:], ps[:], cost[:, sl],
                                                mybir.AluOpType.mult)
                        for h2 in (0, 64):
                            nc.vector.tensor_tensor(
                                dst[h2:h2 + 32, sl], ps[h2 + 32:h2 + 64, :],
                                sint[h2:h2 + 32, sl], mybir.AluOpType.mult)
                            nc.vector.tensor_tensor(
                                dst[h2 + 32:h2 + 64, sl], ps[h2:h2 + 32, :],
                                sint[h2 + 32:h2 + 64, sl], mybir.AluOpType.mult)
                        nc.vector.tensor_tensor(dst[:, sl], dst[:, sl], tcs[:],
                                                mybir.AluOpType.add)
                    # v group: d-major matmul, then PE-transpose to s-major
                    psw = psb.tile([P, 2 * F], f32, tag="spair", bufs=2,
                                   name="pswv")
                    ps = psw[:, 0:F]
                    for dc in range(8):
                        nc.tensor.matmul(
                            ps[:], whp[:, dc, 2 * P:3 * P],
                            xts[:, dc, :], start=(dc == 0), stop=(dc == 7),
                        )
                    vdm = tmpp.tile([P, F], f32, tag="vdm")
                    nc.vector.tensor_copy(vdm[:], ps[:])
                    for j in range(4):
                        kt = sc * 4 + j
                        pt = pss.tile([P, P], f32, tag="small")
                        nc.tensor.transpose(pt[:], vdm[:, j * P:(j + 1) * P],
                                            ident[:])
                        nc.vector.tensor_copy(v_sb[:, kt, 0:64], pt[:, 0:64])
                        nc.vector.tensor_copy(v_sb[:, kt, 65:129], pt[:, 64:128])

                # ---- causal attention for this head pair ----
                for qc in range(N_SC):
                    qsl = slice(qc * F, (qc + 1) * F)
                    po = [psb.tile([P, F], f32, tag="po", bufs=2,
                                   name=f"po{h2}")
                          for h2 in range(2)]
                    nkt = 4 * qc + 4
                    for kp in range(nkt // 2):
                        exs = []
                        for h2 in (0, 1):
                            base = 64 * h2
                            ps = psb.tile([P, 2 * F], f32, tag="spair",
                                          bufs=2, name="spair")
                            for s2 in (0, 1):
                                kt = 2 * kp + s2
                                fsl = slice(s2 * F, (s2 + 1) * F)
                                nc.tensor.matmul(
                                    ps[:, fsl],
                                    k_rot[base:base + 64, kt * P:(kt + 1) * P],
                                    q_rot[base:base + 64, qsl],
                                    start=True, stop=(kt < 4 * qc),
                                )
                                if kt >= 4 * qc:
                                    nc.tensor.matmul(
                                        ps[:, fsl], identc[:],
                                        maskt[:, kt - 4 * qc, :],
                                        start=False, stop=True,
                                    )
                            ex = expp.tile([P, 2 * F], cdt)
                            nc.scalar.activation(
                                ex[:], ps[:], mybir.ActivationFunctionType.Exp,
                                scale=1.0 / math.sqrt(DH))
                            exs.append(ex)
                        for h2 in (0, 1):
                            for s2 in (0, 1):
                                kt = 2 * kp + s2
                                nc.tensor.matmul(
                                    po[h2][0:65, :],
                                    v_sb[:, kt, 65 * h2:65 * h2 + 65],
                                    exs[h2][:, s2 * F:(s2 + 1) * F],
                                    start=(kt == 0), stop=(kt == nkt - 1),
                                )
                    for h2 in (0, 1):
                        nc.vector.tensor_copy(
                            outt[64 * h2:64 * h2 + 64, hp, qsl],
                            po[h2][0:64, :])
                        h = 2 * hp + h2
                        rsX = rsA if h < 4 else rsB
                        row = 32 * (h % 4)
                        nc.vector.tensor_copy(rsX[row:row + 1, qsl],
                                              po[h2][64:65, :])

            # ---- denominators: transpose, reciprocal, broadcast, divide ----
            for j in range(N_QT):
                for half, rsX in ((0, rsA), (1, rsB)):
                    pt = pss.tile([P, P], f32, tag="small")
                    nc.tensor.transpose(pt[:], rsX[:, j * P:(j + 1) * P],
                                        ident[:])
                    for hh in range(4):
                        h = 4 * half + hh
                        nc.vector.reciprocal(rcp[:, j, h:h + 1],
                                             pt[:, 32 * hh:32 * hh + 1])
            for hp in range(HPAIRS):
                for j in range(N_QT):
                    jsl = slice(j * P, (j + 1) * P)
                    for h2 in (0, 1):
                        h = 2 * hp + h2
                        diag = smallp.tile([P, P], f32, tag="diag")
                        nc.vector.tensor_scalar(
                            diag[:], ident[:], rcp[:, j, h:h + 1], None,
                            mybir.AluOpType.mult)
                        pbc = pss.tile([P, P], f32, tag="small")
                        nc.tensor.matmul(pbc[0:64, :], ones64[:, 0:64],
                                         diag[:], start=True, stop=True)
                        nc.vector.tensor_tensor(
                            outt[64 * h2:64 * h2 + 64, hp, jsl],
                            outt[64 * h2:64 * h2 + 64, hp, jsl],
                            pbc[0:64, :], mybir.AluOpType.mult)

            # ---- output projection: natural [s, e] partial ----
            for ec in range(2):
                esl = slice(ec * F, (ec + 1) * F)
                for st in range(N_QT):
                    pfw = psb.tile([P, 2 * F], f32, tag="spair", bufs=2,
                                   name="pfw")
                    pf = pfw[:, 0:F]
                    for hp in range(HPAIRS):
                        nc.tensor.matmul(
                            pf[:], outt[:, hp, st * P:(st + 1) * P],
                            woutt[:, hp, esl],
                            start=(hp == 0), stop=(hp == 3),
                        )
                    fo = finp.tile([P, F], f32)
                    nc.scalar.copy(fo[:], pf[:])
                    nc.sync.dma_start(
                        out_d.ap()[st * P:(st + 1) * P, esl], fo[:])

    nc.compile()
    return nc


def _rope_tables():
    k = np.arange(DH // 2, dtype=np.float64)
    invf = THETA ** (-2.0 * k / DH)
    pos = np.arange(S, dtype=np.float64)
    ang = invf[:, None] * pos[None, :]  # [32, S]
    cos32 = np.cos(ang)
    sin32 = np.sin(ang)
    cos = np.tile(cos32, (4, 1)).astype(np.float32)          # [128, S]
    sins = np.concatenate([-sin32, sin32, -sin32, sin32], 0).astype(np.float32)
    return cos, sins


def _masks():
    m = np.arange(4)[:, None, None]
    i = np.arange(P)[None, :, None]
    j = np.arange(F)[None, None, :]
    return np.where(P * m + i > j, np.float32(NEG), np.float32(0.0))


def host_inputs(x, Wqkv, Wout, core):
    """Per-core input dict (all fp32 contiguous)."""
    b, g = core // 2, core % 2
    xt = np.ascontiguousarray(x[b].T, dtype=np.float32)  # [1024, 2048]
    perm = np.concatenate([np.arange(0, DH, 2), np.arange(1, DH, 2)])
    blocks = []
    for hp in range(HPAIRS):
        hA = 8 * g + 2 * hp
        for off, do_perm in ((0, True), (D_MODEL, True), (2 * D_MODEL, False)):
            for h in (hA, hA + 1):
                rows = Wqkv[off + h * DH: off + (h + 1) * DH]
                if do_perm:
                    rows = rows[perm]
                blocks.append(rows)
    wq = np.ascontiguousarray(np.concatenate(blocks, 0).T, dtype=np.float32)
    wo = np.ascontiguousarray(Wout[:, 512 * g:512 * (g + 1)].T,
                              dtype=np.float32)
    cos, sins = _rope_tables()
    return {"xt": xt, "wqkv": wq, "wout": wo, "costab": cos, "sintab": sins,
            "masks": _masks()}


_CACHE = {}


def kernel(x, Wqkv, Wout):
    from concourse.bass_utils import run_bass_kernel_spmd

    x = np.asarray(x, dtype=np.float32)
    Wqkv = np.asarray(Wqkv, dtype=np.float32)
    Wout = np.asarray(Wout, dtype=np.float32)

    if "nc" not in _CACHE:
        _CACHE["nc"] = build_program(debug=False)
    nc = _CACHE["nc"]

    in_maps = [host_inputs(x, Wqkv, Wout, c) for c in range(N_CORES)]
    res = run_bass_kernel_spmd(nc, in_maps, list(range(N_CORES))).results
    out = np.empty((B, S, D_MODEL), dtype=np.float32)
    for b in range(B):
        out[b] = res[2 * b]["out"] + res[2 * b + 1]["out"]
    return out


# revision 15
# speedup vs baseline: 1.6421x; 1.4411x over previous
"""Causal multi-head attention with RoPE on 8 Trainium2 NeuronCores.

Reference computation (fp32):
    qkv = x @ Wqkv.T ; split q,k,v ; heads 16 x 64 ; interleaved-pair RoPE on
    q,k ; causal softmax(q k^T / 8) @ v ; concat heads ; out @ Wout.T

Sharding: core c -> batch b=c//2, head-group g=c%2 (heads 8g..8g+8).
Each core computes a [2048, 1024] partial of the output projection for its
batch (contraction over its 512 head-dims); host sums core pairs.

Kernel-internal layout tricks:
  - Wqkv rows per head are permuted evens-then-odds so RoPE becomes
    block-wise (no interleaving on device). The same permutation applied to
    q and k leaves q.k^T invariant.
  - Scores are computed transposed (S^T[k, q]) so the PV matmul needs no
    transposes; PV uses a ones-augmented V (M=65) so row 64 of the PV psum
    accumulates the softmax denominator for free.
  - Causal masks are added into the scores psum by an accumulating
    identity @ mask matmul on the PE (keeps DVE free, keeps PE warm).
  - Denominators are transposed via the PE, reciprocated in fp32, and
    broadcast across partitions with a ones @ diag(recip) matmul; the
    division is one elementwise multiply per output tile.

Matmul dtype MM_DT (env): bfloat16 (default, host pre-rounds inputs),
float32r, or float32. The softmax denominator / division chain is fp32
in all modes.
"""

import math
import os
import sys

import numpy as np

sys.path.insert(0, "/opt/trn_rl_repo")

import concourse.bass as bass  # noqa: E402,F401  (re-exported for tooling)
import concourse.mybir as mybir  # noqa: E402
from concourse import bacc, tile  # noqa: E402
from concourse.masks import make_identity  # noqa: E402

D_MODEL = 1024
NUM_HEADS = 16
DH = 64
S = 2048
B = 4
THETA = 10000.0
P = 128
N_CORES = 8
F = 512  # free-dim chunk
N_SC = S // F  # 4 s-chunks
N_QT = S // P  # 16 q-tiles of 128
HPAIRS = 4  # head pairs per core
NEG = -1.0e30

MM_DT = getattr(mybir.dt, os.environ.get("MM_DT", "bfloat16"))


def build_program(debug: bool = False):
    """Build the single-core SPMD program (identical on all 8 cores)."""
    nc = bacc.Bacc("TRN2", target_bir_lowering=False, debug=debug,
                   enable_asserts=debug)
    f32 = mybir.dt.float32
    cdt = MM_DT

    xt_d = nc.dram_tensor("xt", [D_MODEL, S], cdt, kind="ExternalInput")
    wq_d = nc.dram_tensor("wqkv", [D_MODEL, 12 * P], cdt, kind="ExternalInput")
    wo_d = nc.dram_tensor("wout", [4 * P, D_MODEL], cdt, kind="ExternalInput")
    cos_d = nc.dram_tensor("costab", [P, S], cdt, kind="ExternalInput")
    sinw_d = nc.dram_tensor("sinswt", [P, S], cdt, kind="ExternalInput")
    mask_d = nc.dram_tensor("masks", [4, P, F], cdt, kind="ExternalInput")
    out_d = nc.dram_tensor("out", [S, D_MODEL], f32, kind="ExternalOutput")

    xt_r = xt_d.ap().rearrange("(dc p) s -> p dc s", p=P)  # [128, 8, 2048]
    wq_r = wq_d.ap().rearrange("(dc p) n -> p dc n", p=P)  # [128, 8, 1536]
    wo_r = wo_d.ap().rearrange("(hp p) e -> p hp e", p=P)  # [128, 4, 1024]
    mask_r = mask_d.ap().rearrange("m p n -> p m n")       # [128, 4, 512]

    with tile.TileContext(nc) as tc:
        with (
            tc.tile_pool(name="const", bufs=1) as const,
            tc.tile_pool(name="wq", bufs=2) as wqp,
            tc.tile_pool(name="qkv", bufs=1) as qkvp,
            tc.tile_pool(name="tmp", bufs=3) as tmpp,
            tc.tile_pool(name="outt", bufs=1) as outtp,
            tc.tile_pool(name="exp", bufs=4) as expp,
            tc.tile_pool(name="fin", bufs=3) as finp,
            tc.tile_pool(name="small", bufs=2) as smallp,
            tc.tile_pool(name="psb", bufs=2, space="PSUM") as psb,
            tc.tile_pool(name="pss", bufs=2, space="PSUM") as pss,
        ):
            # ---- constants ----
            ident = const.tile([P, P], f32)
            make_identity(nc, ident)
            identc = const.tile([P, P], cdt)
            nc.vector.tensor_copy(identc[:], ident[:])
            ones64 = const.tile([P, 64], f32)
            nc.vector.memset(ones64[:], 1.0)
            cost = const.tile([P, S], cdt)
            nc.sync.dma_start(cost[:], cos_d.ap())
            sinw = const.tile([P, S], cdt)
            nc.sync.dma_start(sinw[:], sinw_d.ap())
            maskt = const.tile([P, 4, F], cdt)
            nc.sync.dma_start(maskt[:], mask_r)
            woutt = const.tile([P, 4, D_MODEL], cdt)
            nc.sync.dma_start(woutt[:], wo_r)
            # x^T resident: [128, dchunk, s]
            xts = const.tile([P, 8, S], cdt)
            nc.sync.dma_start(xts[:], xt_r)
            # attention output (d-major), all 4 head pairs: rows=[hA|hB] dims
            outt = outtp.tile([P, HPAIRS, S], cdt)
            # softmax denominators: heads 0..3 on rows {0,32,64,96} of rsA,
            # heads 4..7 on rsB (engine APs need 32-aligned start partitions)
            rsA = const.tile([P, S], f32, name="rsA")
            rsB = const.tile([P, S], f32, name="rsB")
            nc.vector.memset(rsA[:], 1.0)
            nc.vector.memset(rsB[:], 1.0)
            # reciprocals, transposed: [q-within-tile, qtile, head]
            rcp = const.tile([P, N_QT, 8], f32)

            for hp in range(HPAIRS):
                whp = wqp.tile([P, 8, 3 * P], cdt)
                nc.sync.dma_start(whp[:], wq_r[:, :, hp * 3 * P:(hp + 1) * 3 * P])
                q_rot = qkvp.tile([P, S], cdt, tag="q_rot")
                k_rot = qkvp.tile([P, S], cdt, tag="k_rot")
                # V s-major + ones cols: [s-part, ktile, (vA|1|vB|1)]
                v_sb = qkvp.tile([P, N_QT, 130], cdt, tag="v_sb")
                nc.vector.tensor_copy(v_sb[:, :, 64:65], ones64[:, 0:N_QT, None])
                nc.vector.tensor_copy(v_sb[:, :, 129:130],
                                      ones64[:, 0:N_QT, None])

                for sc in range(N_SC):
                    sl = slice(sc * F, (sc + 1) * F)
                    # q and k groups (d-major); psum evac to sbuf via ACT,
                    # then RoPE on DVE in the compute dtype (2x mode for bf16)
                    for gi, dst in ((0, q_rot), (1, k_rot)):
                        psw = psb.tile([P, 2 * F], f32, tag="spair", bufs=2,
                                       name="psw")
                        ps = psw[:, 0:F]
                        for dc in range(8):
                            nc.tensor.matmul(
                                ps,
                                whp[:, dc, gi * P:(gi + 1) * P],
                                xts[:, dc, sl],
                                start=(dc == 0), stop=(dc == 7),
                            )
                        qk = tmpp.tile([P, F], cdt, tag="qk_sb")
                        nc.scalar.copy(qk[:], ps)
                        # rot = qk*cos + swap_within_head(qk)*sins
                        tcs = tmpp.tile([P, F], cdt, tag="ropetmp")
                        nc.vector.tensor_tensor(tcs[:], qk[:], cost[:, sl],
                                                mybir.AluOpType.mult)
                        for h2 in (0, 64):
                            nc.vector.tensor_tensor(
                                dst[h2:h2 + 32, sl], qk[h2 + 32:h2 + 64, :],
                                sinw[h2 + 32:h2 + 64, sl],
                                mybir.AluOpType.mult)
                            nc.vector.tensor_tensor(
                                dst[h2 + 32:h2 + 64, sl], qk[h2:h2 + 32, :],
                                sinw[h2:h2 + 32, sl], mybir.AluOpType.mult)
                        nc.vector.tensor_tensor(dst[:, sl], dst[:, sl], tcs[:],
                                                mybir.AluOpType.add)
                    # v group: d-major matmul, then PE-transpose to s-major
                    psw = psb.tile([P, 2 * F], f32, tag="spair", bufs=2,
                                   name="pswv")
                    ps = psw[:, 0:F]
                    for dc in range(8):
                        nc.tensor.matmul(
                            ps, whp[:, dc, 2 * P:3 * P],
                            xts[:, dc, sl], start=(dc == 0), stop=(dc == 7),
                        )
                    vdm = tmpp.tile([P, F], cdt, tag="vdm")
                    nc.scalar.copy(vdm[:], ps)
                    for j in range(4):
                        kt = sc * 4 + j
                        pt = pss.tile([P, P], cdt, tag="small")
                        nc.tensor.transpose(pt[:], vdm[:, j * P:(j + 1) * P],
                                            identc[:])
                        nc.scalar.copy(v_sb[:, kt, 0:64], pt[:, 0:64])
                        nc.scalar.copy(v_sb[:, kt, 65:129], pt[:, 64:128])

                # ---- causal attention for this head pair ----
                for qc in range(N_SC):
                    qsl = slice(qc * F, (qc + 1) * F)
                    po = [psb.tile([P, F], f32, tag="po", bufs=2,
                                   name=f"po{h2}")
                          for h2 in range(2)]
                    nkt = 4 * qc + 4
                    for kp in range(nkt // 2):
                        exs = []
                        for h2 in (0, 1):
                            base = 64 * h2
                            ps = psb.tile([P, 2 * F], f32, tag="spair",
                                          bufs=2, name="spair")
                            for s2 in (0, 1):
                                kt = 2 * kp + s2
                                fsl = slice(s2 * F, (s2 + 1) * F)
                                nc.tensor.matmul(
                                    ps[:, fsl],
                                    k_rot[base:base + 64, kt * P:(kt + 1) * P],
                                    q_rot[base:base + 64, qsl],
                                    start=True, stop=(kt < 4 * qc),
                                )
                                if kt >= 4 * qc:
                                    nc.tensor.matmul(
                                        ps[:, fsl], identc[:],
                                        maskt[:, kt - 4 * qc, :],
                                        start=False, stop=True,
                                    )
                            ex = expp.tile([P, 2 * F], cdt)
                            nc.scalar.activation(
                                ex[:], ps[:], mybir.ActivationFunctionType.Exp,
                                scale=1.0 / math.sqrt(DH))
                            exs.append(ex)
                        for h2 in (0, 1):
                            for s2 in (0, 1):
                                kt = 2 * kp + s2
                                nc.tensor.matmul(
                                    po[h2][0:65, :],
                                    v_sb[:, kt, 65 * h2:65 * h2 + 65],
                                    exs[h2][:, s2 * F:(s2 + 1) * F],
                                    start=(kt == 0), stop=(kt == nkt - 1),
                                )
                    for h2 in (0, 1):
                        nc.vector.tensor_copy(
                            outt[64 * h2:64 * h2 + 64, hp, qsl],
                            po[h2][0:64, :])
                        h = 2 * hp + h2
                        rsX = rsA if h < 4 else rsB
                        row = 32 * (h % 4)
                        nc.vector.tensor_copy(rsX[row:row + 1, qsl],
                                              po[h2][64:65, :])

            # ---- denominators: transpose, reciprocal, broadcast, divide ----
            for j in range(N_QT):
                for half, rsX in ((0, rsA), (1, rsB)):
                    pt = pss.tile([P, P], f32, tag="small")
                    nc.tensor.transpose(pt[:], rsX[:, j * P:(j + 1) * P],
                                        ident[:])
                    for hh in range(4):
                        h = 4 * half + hh
                        nc.vector.reciprocal(rcp[:, j, h:h + 1],
                                             pt[:, 32 * hh:32 * hh + 1])
            for hp in range(HPAIRS):
                for j in range(N_QT):
                    jsl = slice(j * P, (j + 1) * P)
                    for h2 in (0, 1):
                        h = 2 * hp + h2
                        diag = smallp.tile([P, P], f32, tag="diag")
                        nc.vector.tensor_scalar(
                            diag[:], ident[:], rcp[:, j, h:h + 1], None,
                            mybir.AluOpType.mult)
                        pbc = pss.tile([P, P], f32, tag="small")
                        nc.tensor.matmul(pbc[0:64, :], ones64[:, 0:64],
                                         diag[:], start=True, stop=True)
                        nc.vector.tensor_tensor(
                            outt[64 * h2:64 * h2 + 64, hp, jsl],
                            outt[64 * h2:64 * h2 + 64, hp, jsl],
                            pbc[0:64, :], mybir.AluOpType.mult)

            # ---- output projection: natural [s, e] partial ----
            for ec in range(2):
                esl = slice(ec * F, (ec + 1) * F)
                for st in range(N_QT):
                    pfw = psb.tile([P, 2 * F], f32, tag="spair", bufs=2,
                                   name="pfw")
                    pf = pfw[:, 0:F]
                    for hp in range(HPAIRS):
                        nc.tensor.matmul(
                            pf, outt[:, hp, st * P:(st + 1) * P],
                            woutt[:, hp, esl],
                            start=(hp == 0), stop=(hp == 3),
                        )
                    fo = finp.tile([P, F], f32)
                    nc.scalar.copy(fo[:], pf)
                    nc.sync.dma_start(
                        out_d.ap()[st * P:(st + 1) * P, esl], fo[:])

    nc.compile()
    return nc


def _rope_tables():
    k = np.arange(DH // 2, dtype=np.float64)
    invf = THETA ** (-2.0 * k / DH)
    pos = np.arange(S, dtype=np.float64)
    ang = invf[:, None] * pos[None, :]  # [32, S]
    cos32 = np.cos(ang)
    sin32 = np.sin(ang)
    cos = np.tile(cos32, (4, 1)).astype(np.float32)          # [128, S]
    sins = np.concatenate([-sin32, sin32, -sin32, sin32], 0).astype(np.float32)
    return cos, sins


def _masks():
    m = np.arange(4)[:, None, None]
    i = np.arange(P)[None, :, None]
    j = np.arange(F)[None, None, :]
    return np.where(P * m + i > j, np.float32(NEG), np.float32(0.0))


def _np_dt():
    if MM_DT == mybir.dt.bfloat16:
        import ml_dtypes
        return np.dtype(ml_dtypes.bfloat16)
    return np.dtype(np.float32)


def host_inputs(x, Wqkv, Wout, core):
    """Per-core input dict (cast to the compute dtype on host)."""
    ndt = _np_dt()
    b, g = core // 2, core % 2
    xt = np.ascontiguousarray(x[b].T).astype(ndt)  # [1024, 2048]
    perm = np.concatenate([np.arange(0, DH, 2), np.arange(1, DH, 2)])
    blocks = []
    for hp in range(HPAIRS):
        hA = 8 * g + 2 * hp
        for off, do_perm in ((0, True), (D_MODEL, True), (2 * D_MODEL, False)):
            for h in (hA, hA + 1):
                rows = Wqkv[off + h * DH: off + (h + 1) * DH]
                if do_perm:
                    rows = rows[perm]
                blocks.append(rows)
    wq = np.ascontiguousarray(np.concatenate(blocks, 0).T).astype(ndt)
    wo = np.ascontiguousarray(Wout[:, 512 * g:512 * (g + 1)].T).astype(ndt)
    cos, sins = _rope_tables()
    return {"xt": xt, "wqkv": wq, "wout": wo,
            "costab": cos.astype(ndt), "sinswt": (-sins).astype(ndt),
            "masks": _masks().astype(ndt)}


_CACHE = {}


def kernel(x, Wqkv, Wout):
    from concourse.bass_utils import run_bass_kernel_spmd

    x = np.asarray(x, dtype=np.float32)
    Wqkv = np.asarray(Wqkv, dtype=np.float32)
    Wout = np.asarray(Wout, dtype=np.float32)

    if "nc" not in _CACHE:
        _CACHE["nc"] = build_program(debug=False)
    nc = _CACHE["nc"]

    in_maps = [host_inputs(x, Wqkv, Wout, c) for c in range(N_CORES)]
    res = run_bass_kernel_spmd(nc, in_maps, list(range(N_CORES))).results
    out = np.empty((B, S, D_MODEL), dtype=np.float32)
    for b in range(B):
        out[b] = res[2 * b]["out"] + res[2 * b + 1]["out"]
    return out


# revision 16
# speedup vs baseline: 1.6884x; 1.0282x over previous
"""Causal multi-head attention with RoPE on 8 Trainium2 NeuronCores.

Reference computation (fp32):
    qkv = x @ Wqkv.T ; split q,k,v ; heads 16 x 64 ; interleaved-pair RoPE on
    q,k ; causal softmax(q k^T / 8) @ v ; concat heads ; out @ Wout.T

Sharding: core c -> batch b=c//2, head-group g=c%2 (heads 8g..8g+8).
Each core computes a [2048, 1024] partial of the output projection for its
batch (contraction over its 512 head-dims); host sums core pairs.

Kernel-internal layout tricks:
  - Wqkv rows per head are permuted evens-then-odds so RoPE becomes
    block-wise (no interleaving on device). The same permutation applied to
    q and k leaves q.k^T invariant.
  - Scores are computed transposed (S^T[k, q]) so the PV matmul needs no
    transposes; PV uses a ones-augmented V (M=65) so row 64 of the PV psum
    accumulates the softmax denominator for free.
  - Causal masks are added into the scores psum by an accumulating
    identity @ mask matmul on the PE (keeps DVE free, keeps PE warm).
  - Denominators are transposed via the PE, reciprocated in fp32, and
    broadcast across partitions with a ones @ diag(recip) matmul; the
    division is one elementwise multiply per output tile.

Matmul dtype MM_DT (env): bfloat16 (default, host pre-rounds inputs),
float32r, or float32. The softmax denominator / division chain is fp32
in all modes.
"""

import math
import os
import sys

import numpy as np

sys.path.insert(0, "/opt/trn_rl_repo")

import concourse.bass as bass  # noqa: E402,F401  (re-exported for tooling)
import concourse.mybir as mybir  # noqa: E402
from concourse import bacc, tile  # noqa: E402
from concourse.masks import make_identity  # noqa: E402

D_MODEL = 1024
NUM_HEADS = 16
DH = 64
S = 2048
B = 4
THETA = 10000.0
P = 128
N_CORES = 8
F = 512  # free-dim chunk
N_SC = S // F  # 4 s-chunks
N_QT = S // P  # 16 q-tiles of 128
HPAIRS = 4  # head pairs per core
NEG = -1.0e30

MM_DT = getattr(mybir.dt, os.environ.get("MM_DT", "bfloat16"))


def build_program(debug: bool = False):
    """Build the single-core SPMD program (identical on all 8 cores)."""
    nc = bacc.Bacc("TRN2", target_bir_lowering=False, debug=debug,
                   enable_asserts=debug)
    f32 = mybir.dt.float32
    cdt = MM_DT

    xt_d = nc.dram_tensor("xt", [D_MODEL, S], cdt, kind="ExternalInput")
    wq_d = nc.dram_tensor("wqkv", [D_MODEL, 12 * P], cdt, kind="ExternalInput")
    wo_d = nc.dram_tensor("wout", [4 * P, D_MODEL], cdt, kind="ExternalInput")
    cos_d = nc.dram_tensor("costab", [P, S], cdt, kind="ExternalInput")
    sinw_d = nc.dram_tensor("sinswt", [P, S], cdt, kind="ExternalInput")
    mask_d = nc.dram_tensor("masks", [4, P, F], cdt, kind="ExternalInput")
    out_d = nc.dram_tensor("out", [S, D_MODEL], f32, kind="ExternalOutput")

    xt_r = xt_d.ap().rearrange("(dc p) s -> p dc s", p=P)  # [128, 8, 2048]
    wq_r = wq_d.ap().rearrange("(dc p) n -> p dc n", p=P)  # [128, 8, 1536]
    wo_r = wo_d.ap().rearrange("(hp p) e -> p hp e", p=P)  # [128, 4, 1024]
    mask_r = mask_d.ap().rearrange("m p n -> p m n")       # [128, 4, 512]

    with tile.TileContext(nc) as tc:
        with (
            tc.tile_pool(name="const", bufs=1) as const,
            tc.tile_pool(name="wq", bufs=2) as wqp,
            tc.tile_pool(name="qkv", bufs=1) as qkvp,
            tc.tile_pool(name="tmp", bufs=3) as tmpp,
            tc.tile_pool(name="outt", bufs=1) as outtp,
            tc.tile_pool(name="exp", bufs=4) as expp,
            tc.tile_pool(name="fin", bufs=3) as finp,
            tc.tile_pool(name="small", bufs=2) as smallp,
            tc.tile_pool(name="psb", bufs=2, space="PSUM") as psb,
            tc.tile_pool(name="pss", bufs=2, space="PSUM") as pss,
        ):
            # ---- constants ----
            ident = const.tile([P, P], f32)
            make_identity(nc, ident)
            identc = const.tile([P, P], cdt)
            nc.vector.tensor_copy(identc[:], ident[:])
            ones64 = const.tile([P, 64], f32)
            nc.vector.memset(ones64[:], 1.0)
            rdt = mybir.dt.float32r if MM_DT != mybir.dt.float32 else f32
            ones64r = const.tile([P, 64], rdt)
            nc.vector.tensor_copy(ones64r[:], ones64[:])
            ident4 = const.tile([P, 4, P], rdt)
            for _i4 in range(4):
                nc.vector.tensor_copy(ident4[:, _i4, :], ident[:])
            cost = const.tile([P, S], cdt)
            nc.sync.dma_start(cost[:], cos_d.ap())
            sinw = const.tile([P, S], cdt)
            nc.sync.dma_start(sinw[:], sinw_d.ap())
            maskt = const.tile([P, 4, F], cdt)
            nc.sync.dma_start(maskt[:], mask_r)
            woutt = const.tile([P, 4, D_MODEL], cdt)
            nc.sync.dma_start(woutt[:], wo_r)
            # x^T resident: [128, dchunk, s]
            xts = const.tile([P, 8, S], cdt)
            nc.sync.dma_start(xts[:], xt_r)
            # attention output (d-major), all 4 head pairs: rows=[hA|hB] dims
            outt = outtp.tile([P, HPAIRS, S], cdt)
            # softmax denominators: heads 0..3 on rows {0,32,64,96} of rsA,
            # heads 4..7 on rsB (engine APs need 32-aligned start partitions)
            rsA = const.tile([P, S], f32, name="rsA")
            rsB = const.tile([P, S], f32, name="rsB")
            nc.vector.memset(rsA[:], 1.0)
            nc.vector.memset(rsB[:], 1.0)
            # reciprocals, transposed: [q-within-tile, qtile, head]
            rcp = const.tile([P, N_QT, 8], f32)

            for hp in range(HPAIRS):
                whp = wqp.tile([P, 8, 3 * P], cdt)
                nc.sync.dma_start(whp[:], wq_r[:, :, hp * 3 * P:(hp + 1) * 3 * P])
                q_rot = qkvp.tile([P, S], cdt, tag="q_rot")
                k_rot = qkvp.tile([P, S], cdt, tag="k_rot")
                # V s-major + ones cols: [s-part, ktile, (vA|1|vB|1)]
                v_sb = qkvp.tile([P, N_QT, 130], cdt, tag="v_sb")
                nc.vector.tensor_copy(v_sb[:, :, 64:65], ones64[:, 0:N_QT, None])
                nc.vector.tensor_copy(v_sb[:, :, 129:130],
                                      ones64[:, 0:N_QT, None])

                for sc in range(N_SC):
                    sl = slice(sc * F, (sc + 1) * F)
                    # q and k groups (d-major); psum evac to sbuf via ACT,
                    # then RoPE on DVE in the compute dtype (2x mode for bf16)
                    for gi, dst in ((0, q_rot), (1, k_rot)):
                        psw = psb.tile([P, 2 * F], f32, tag="spair", bufs=2,
                                       name="psw")
                        ps = psw[:, 0:F]
                        for dc in range(8):
                            nc.tensor.matmul(
                                ps,
                                whp[:, dc, gi * P:(gi + 1) * P],
                                xts[:, dc, sl],
                                start=(dc == 0), stop=(dc == 7),
                            )
                        qk = tmpp.tile([P, F], cdt, tag="qk_sb")
                        nc.scalar.copy(qk[:], ps)
                        # rot = qk*cos + swap_within_head(qk)*sins
                        tcs = tmpp.tile([P, F], cdt, tag="ropetmp")
                        nc.vector.tensor_tensor(tcs[:], qk[:], cost[:, sl],
                                                mybir.AluOpType.mult)
                        for h2 in (0, 64):
                            nc.vector.tensor_tensor(
                                dst[h2:h2 + 32, sl], qk[h2 + 32:h2 + 64, :],
                                sinw[h2 + 32:h2 + 64, sl],
                                mybir.AluOpType.mult)
                            nc.vector.tensor_tensor(
                                dst[h2 + 32:h2 + 64, sl], qk[h2:h2 + 32, :],
                                sinw[h2:h2 + 32, sl], mybir.AluOpType.mult)
                        nc.vector.tensor_tensor(dst[:, sl], dst[:, sl], tcs[:],
                                                mybir.AluOpType.add)
                    # v group: d-major matmul, then PE-transpose to s-major
                    psw = psb.tile([P, 2 * F], f32, tag="spair", bufs=2,
                                   name="pswv")
                    ps = psw[:, 0:F]
                    for dc in range(8):
                        nc.tensor.matmul(
                            ps, whp[:, dc, 2 * P:3 * P],
                            xts[:, dc, sl], start=(dc == 0), stop=(dc == 7),
                        )
                    vdm = tmpp.tile([P, F], cdt, tag="vdm")
                    nc.scalar.copy(vdm[:], ps)
                    for j in range(4):
                        kt = sc * 4 + j
                        pt = pss.tile([P, P], cdt, tag="small")
                        nc.tensor.transpose(pt[:], vdm[:, j * P:(j + 1) * P],
                                            identc[:])
                        nc.vector.tensor_copy(v_sb[:, kt, 0:64], pt[:, 0:64])
                        nc.vector.tensor_copy(v_sb[:, kt, 65:129],
                                              pt[:, 64:128])

                # ---- causal attention for this head pair ----
                for qc in range(N_SC):
                    qsl = slice(qc * F, (qc + 1) * F)
                    po = [psb.tile([P, F], f32, tag="po", bufs=2,
                                   name=f"po{h2}")
                          for h2 in range(2)]
                    nkt = 4 * qc + 4
                    for kp in range(nkt // 2):
                        exs = []
                        pss_pair = []
                        for h2 in (0, 1):
                            pss_pair.append(psb.tile([P, 2 * F], f32,
                                                     tag="spair", bufs=2,
                                                     name=f"spair{h2}"))
                        # head A and B score matmuls adjacent per k-tile:
                        # disjoint row groups (0:64 / 64:128) run concurrently
                        for s2 in (0, 1):
                            kt = 2 * kp + s2
                            fsl = slice(s2 * F, (s2 + 1) * F)
                            for h2 in (0, 1):
                                base = 64 * h2
                                nc.tensor.matmul(
                                    pss_pair[h2][:, fsl],
                                    k_rot[base:base + 64, kt * P:(kt + 1) * P],
                                    q_rot[base:base + 64, qsl],
                                    start=True, stop=(kt < 4 * qc),
                                )
                        for s2 in (0, 1):
                            kt = 2 * kp + s2
                            fsl = slice(s2 * F, (s2 + 1) * F)
                            if kt >= 4 * qc:
                                for h2 in (0, 1):
                                    nc.tensor.matmul(
                                        pss_pair[h2][:, fsl], identc[:],
                                        maskt[:, kt - 4 * qc, :],
                                        start=False, stop=True,
                                    )
                        for h2 in (0, 1):
                            ex = expp.tile([P, 2 * F], cdt)
                            nc.scalar.activation(
                                ex[:], pss_pair[h2][:],
                                mybir.ActivationFunctionType.Exp,
                                scale=1.0 / math.sqrt(DH))
                            exs.append(ex)
                        for h2 in (0, 1):
                            for s2 in (0, 1):
                                kt = 2 * kp + s2
                                nc.tensor.matmul(
                                    po[h2][0:65, :],
                                    v_sb[:, kt, 65 * h2:65 * h2 + 65],
                                    exs[h2][:, s2 * F:(s2 + 1) * F],
                                    start=(kt == 0), stop=(kt == nkt - 1),
                                )
                    for h2 in (0, 1):
                        nc.vector.tensor_copy(
                            outt[64 * h2:64 * h2 + 64, hp, qsl],
                            po[h2][0:64, :])
                        h = 2 * hp + h2
                        rsX = rsA if h < 4 else rsB
                        row = 32 * (h % 4)
                        nc.vector.tensor_copy(rsX[row:row + 1, qsl],
                                              po[h2][64:65, :])

            # ---- denominators: transpose, reciprocal, broadcast, divide ----
            for j in range(N_QT):
                for half, rsX in ((0, rsA), (1, rsB)):
                    pt = pss.tile([P, P], f32, tag="small")
                    nc.tensor.transpose(pt[:], rsX[:, j * P:(j + 1) * P],
                                        ident[:])
                    for hh in range(4):
                        h = 4 * half + hh
                        nc.vector.reciprocal(rcp[:, j, h:h + 1],
                                             pt[:, 32 * hh:32 * hh + 1])
            for hp in range(HPAIRS):
                for qc in range(N_SC):
                    qsl = slice(qc * F, (qc + 1) * F)
                    for h2 in (0, 1):
                        h = 2 * hp + h2
                        diag4 = smallp.tile([P, 4, P], rdt, tag="diag")
                        nc.vector.tensor_tensor(
                            diag4[:], ident4[:],
                            rcp[:, 4 * qc:4 * qc + 4, h:h + 1].to_broadcast(
                                (P, 4, P)),
                            mybir.AluOpType.mult)
                        pbcw = psb.tile([P, 2 * F], f32, tag="spair", bufs=2,
                                        name="pbcw")
                        pbc = pbcw[:, 0:F]
                        nc.tensor.matmul(pbc[0:64, :], ones64r[:, 0:64],
                                         diag4[:], start=True, stop=True)
                        nc.vector.tensor_tensor(
                            outt[64 * h2:64 * h2 + 64, hp, qsl],
                            outt[64 * h2:64 * h2 + 64, hp, qsl],
                            pbc[0:64, :], mybir.AluOpType.mult)

            # ---- output projection: natural [s, e] partial ----
            for ec in range(2):
                esl = slice(ec * F, (ec + 1) * F)
                for st in range(N_QT):
                    pfw = psb.tile([P, 2 * F], f32, tag="spair", bufs=2,
                                   name="pfw")
                    pf = pfw[:, 0:F]
                    for hp in range(HPAIRS):
                        nc.tensor.matmul(
                            pf, outt[:, hp, st * P:(st + 1) * P],
                            woutt[:, hp, esl],
                            start=(hp == 0), stop=(hp == 3),
                        )
                    fo = finp.tile([P, F], f32)
                    nc.scalar.copy(fo[:], pf)
                    nc.sync.dma_start(
                        out_d.ap()[st * P:(st + 1) * P, esl], fo[:])

    nc.compile()
    return nc


def _rope_tables():
    k = np.arange(DH // 2, dtype=np.float64)
    invf = THETA ** (-2.0 * k / DH)
    pos = np.arange(S, dtype=np.float64)
    ang = invf[:, None] * pos[None, :]  # [32, S]
    cos32 = np.cos(ang)
    sin32 = np.sin(ang)
    cos = np.tile(cos32, (4, 1)).astype(np.float32)          # [128, S]
    sins = np.concatenate([-sin32, sin32, -sin32, sin32], 0).astype(np.float32)
    return cos, sins


def _masks():
    m = np.arange(4)[:, None, None]
    i = np.arange(P)[None, :, None]
    j = np.arange(F)[None, None, :]
    return np.where(P * m + i > j, np.float32(NEG), np.float32(0.0))


def _np_dt():
    if MM_DT == mybir.dt.bfloat16:
        import ml_dtypes
        return np.dtype(ml_dtypes.bfloat16)
    return np.dtype(np.float32)


def host_inputs(x, Wqkv, Wout, core):
    """Per-core input dict (cast to the compute dtype on host)."""
    ndt = _np_dt()
    b, g = core // 2, core % 2
    xt = np.ascontiguousarray(x[b].T).astype(ndt)  # [1024, 2048]
    perm = np.concatenate([np.arange(0, DH, 2), np.arange(1, DH, 2)])
    blocks = []
    for hp in range(HPAIRS):
        hA = 8 * g + 2 * hp
        for off, do_perm in ((0, True), (D_MODEL, True), (2 * D_MODEL, False)):
            for h in (hA, hA + 1):
                rows = Wqkv[off + h * DH: off + (h + 1) * DH]
                if do_perm:
                    rows = rows[perm]
                blocks.append(rows)
    wq = np.ascontiguousarray(np.concatenate(blocks, 0).T).astype(ndt)
    wo = np.ascontiguousarray(Wout[:, 512 * g:512 * (g + 1)].T).astype(ndt)
    cos, sins = _rope_tables()
    return {"xt": xt, "wqkv": wq, "wout": wo,
            "costab": cos.astype(ndt), "sinswt": (-sins).astype(ndt),
            "masks": _masks().astype(ndt)}


_CACHE = {}


def kernel(x, Wqkv, Wout):
    from concourse.bass_utils import run_bass_kernel_spmd

    x = np.asarray(x, dtype=np.float32)
    Wqkv = np.asarray(Wqkv, dtype=np.float32)
    Wout = np.asarray(Wout, dtype=np.float32)

    if "nc" not in _CACHE:
        _CACHE["nc"] = build_program(debug=False)
    nc = _CACHE["nc"]

    in_maps = [host_inputs(x, Wqkv, Wout, c) for c in range(N_CORES)]
    res = run_bass_kernel_spmd(nc, in_maps, list(range(N_CORES))).results
    out = np.empty((B, S, D_MODEL), dtype=np.float32)
    for b in range(B):
        out[b] = res[2 * b]["out"] + res[2 * b + 1]["out"]
    return out
